# revision 1
# baseline (speedup 1.0000x reference)
"""Gated multi-head self-attention on 8 Trainium2 NeuronCores.

Sharding: batch (B=2) x head-groups (4 groups of 4 heads) -> 8 cores.
Each core computes, for its batch b and its 4 heads:
    partial_out[t, e] = sum_h gate[h] * (softmax(Q_h K_h^T / 8) (V_h + bv_h) Wo_h + bo_h)
The host sums the 4 head-group partials per batch (the "all-reduce") and
stacks the two batches.

Per-core dataflow (all matmuls in float32r = full-rate fp32, ~1.5e-4 rel):
  hT [E, T] (host-transposed)  --PE-->  QT/KT [128(2x64d), T] per head-pair
                               --PE-->  V [T, 256(4x64d)] (+bias via K=1 matmul)
  scoresT[s,t] = K^T Q per head (row-tiled pairs)  --ACT--> exp (bf16)
  rowsum via DVE chain-add + ones-matmul; PV col-tiled pairs -> ctxT
  ctxT/rowsum (DVE) --PE row-tiled--> out[t, e] += bias (K=1 matmul)
"""

import numpy as np
import ml_dtypes
from contextlib import ExitStack

import concourse.bass as bass
import concourse.tile as tile
from concourse import bacc, mybir
from concourse import bass_utils

E, H, D = 1024, 16, 64
B, T = 2, 2048
NCORES = 8
P = 128
TC = 512          # t-chunk (PSUM bank = 512 fp32)
NTC = T // TC     # 4 t-chunks
NST = T // P      # 16 s-tiles
NEC = E // P      # 8 e-chunks

F32 = mybir.dt.float32
F32R = mybir.dt.float32r
BF16 = mybir.dt.bfloat16


def build_kernel():
    nc = bacc.Bacc("TRN2", target_bir_lowering=False, debug=False,
                   num_devices=NCORES)
    hT = nc.dram_tensor("hT", [NEC, P, T], F32R, kind="ExternalInput").ap()
    wq = nc.dram_tensor("wq", [2, P, NEC, P], F32R, kind="ExternalInput").ap()
    wk = nc.dram_tensor("wk", [2, P, NEC, P], F32R, kind="ExternalInput").ap()
    wv = nc.dram_tensor("wv", [P, NEC, 256], F32R, kind="ExternalInput").ap()
    wo = nc.dram_tensor("wo", [2, P, E], F32R, kind="ExternalInput").ap()
    bq = nc.dram_tensor("bq", [2, 1, P], F32R, kind="ExternalInput").ap()
    bk = nc.dram_tensor("bk", [2, 1, P], F32R, kind="ExternalInput").ap()
    bv = nc.dram_tensor("bv", [1, 256], F32R, kind="ExternalInput").ap()
    bo = nc.dram_tensor("bo", [1, E], F32R, kind="ExternalInput").ap()
    ones_r = nc.dram_tensor("ones_r", [1, TC], F32R, kind="ExternalInput").ap()
    ones_b = nc.dram_tensor("ones_b", [P, 1], BF16, kind="ExternalInput").ap()
    sel = nc.dram_tensor("sel", [1, 2 * P], F32R, kind="ExternalInput").ap()
    out = nc.dram_tensor("out", [T, E], F32, kind="ExternalOutput").ap()

    with tile.TileContext(nc) as tc:
        with ExitStack() as ctx:
            persist = ctx.enter_context(tc.tile_pool(name="persist", bufs=1))
            work = ctx.enter_context(tc.tile_pool(name="work", bufs=4))
            rspool = ctx.enter_context(tc.tile_pool(name="rspool", bufs=2))
            ps_s = ctx.enter_context(tc.tile_pool(name="ps_s", bufs=2, space="PSUM"))
            ps_ctx = ctx.enter_context(tc.tile_pool(name="ps_ctx", bufs=2, space="PSUM"))
            ps_misc = ctx.enter_context(tc.tile_pool(name="ps_misc", bufs=2, space="PSUM"))

            # ---- persistent SBUF tensors ----
            hT_sb = persist.tile([P, NEC, T], F32R, tag="hT")
            wq_sb = persist.tile([P, 2, NEC, P], F32R, tag="wq")
            wk_sb = persist.tile([P, 2, NEC, P], F32R, tag="wk")
            wv_sb = persist.tile([P, NEC, 256], F32R, tag="wv")
            wo_sb = persist.tile([P, 2, E], F32R, tag="wo")
            bq_sb = persist.tile([1, 2, P], F32R, tag="bq")
            bk_sb = persist.tile([1, 2, P], F32R, tag="bk")
            bv_sb = persist.tile([1, 256], F32R, tag="bv")
            bo_sb = persist.tile([1, E], F32R, tag="bo")
            on_r = persist.tile([1, TC], F32R, tag="on_r")
            on_b = persist.tile([P, 1], BF16, tag="on_b")
            sel_sb = persist.tile([1, 2 * P], F32R, tag="sel")
            QT_sb = persist.tile([P, 2, T], F32R, tag="QT")
            KT_sb = persist.tile([P, 2, T], F32R, tag="KT")
            V_sb = persist.tile([P, NST, 256], BF16, tag="V")
            ctx_sb = persist.tile([P, 2, T], F32R, tag="ctx")

            with nc.named_scope("load"):
                for ec in range(NEC):
                    nc.sync.dma_start(hT_sb[:, ec, :], hT[ec])
                nc.sync.dma_start(wq_sb[:], wq.rearrange("a p c d -> p a c d"))
                nc.sync.dma_start(wk_sb[:], wk.rearrange("a p c d -> p a c d"))
                nc.sync.dma_start(wv_sb[:], wv)
                nc.sync.dma_start(wo_sb[:], wo.rearrange("a p e -> p a e"))
                nc.sync.dma_start(bq_sb[:], bq.rearrange("a o p -> o a p"))
                nc.sync.dma_start(bk_sb[:], bk.rearrange("a o p -> o a p"))
                nc.sync.dma_start(bv_sb[:], bv)
                nc.sync.dma_start(bo_sb[:], bo)
                nc.sync.dma_start(on_r[:], ones_r)
                nc.sync.dma_start(on_b[:], ones_b)
                nc.sync.dma_start(sel_sb[:], sel)

            # ---- phase 1: QKV projections ----
            with nc.named_scope("qkv"):
                for pr in range(2):
                    for (w_sb, b_sb, dst) in ((wq_sb, bq_sb, QT_sb), (wk_sb, bk_sb, KT_sb)):
                        for tch in range(NTC):
                            ps = ps_misc.tile([P, TC], F32, tag="ps_misc")
                            for ec in range(NEC):
                                nc.tensor.matmul(
                                    ps[:], w_sb[:, pr, ec, :],
                                    hT_sb[:, ec, tch * TC:(tch + 1) * TC],
                                    start=(ec == 0), stop=False)
                            nc.tensor.matmul(ps[:], b_sb[:, pr, :], on_r[:],
                                             start=False, stop=True)
                            nc.vector.tensor_copy(
                                dst[:, pr, tch * TC:(tch + 1) * TC], ps[:])
                for st in range(NST):
                    ps = ps_misc.tile([P, TC], F32, tag="ps_misc")
                    psv = ps[:, :256]
                    for ec in range(NEC):
                        nc.tensor.matmul(
                            psv, hT_sb[:, ec, st * P:(st + 1) * P],
                            wv_sb[:, ec, :], start=(ec == 0), stop=False)
                    nc.tensor.matmul(psv, on_r[:1, :P], bv_sb[:],
                                     start=False, stop=True)
                    nc.vector.tensor_copy(V_sb[:, st, :], psv)

            # ---- phase 2: attention ----
            with nc.named_scope("attn"):
                for tch in range(NTC):
                    t0 = tch * TC
                    for pr in range(2):
                        pctx = ps_ctx.tile([P, TC], F32, tag="ps_ctx")
                        rs = rspool.tile([P, 2 * TC], BF16, tag="rs")
                        for st in range(NST):
                            s0 = st * P
                            pss = ps_s.tile([P, 2 * TC], F32, tag="ps_s")
                            nc.tensor.matmul(
                                pss[:, :TC], KT_sb[0:64, pr, s0:s0 + P],
                                QT_sb[0:64, pr, t0:t0 + TC],
                                start=True, stop=True, tile_position=(0, 0))
                            nc.tensor.matmul(
                                pss[:, TC:], KT_sb[64:P, pr, s0:s0 + P],
                                QT_sb[64:P, pr, t0:t0 + TC],
                                start=True, stop=True, tile_position=(64, 0))
                            ex = work.tile([P, 2 * TC], BF16, tag="expT")
                            nc.scalar.activation(
                                ex[:], pss[:],
                                mybir.ActivationFunctionType.Exp, scale=0.125)
                            if st == 0:
                                nc.vector.tensor_copy(rs[:], ex[:])
                            else:
                                nc.vector.tensor_add(rs[:], rs[:], ex[:])
                            c0 = pr * P
                            nc.tensor.matmul(
                                pctx[0:64, :], V_sb[:, st, c0:c0 + 64],
                                ex[:, :TC],
                                start=(st == 0), stop=(st == NST - 1),
                                tile_position=(0, 0), skip_group_check=True)
                            nc.tensor.matmul(
                                pctx[64:P, :], V_sb[:, st, c0 + 64:c0 + P],
                                ex[:, TC:],
                                start=(st == 0), stop=(st == NST - 1),
                                tile_position=(0, 64), skip_group_check=True)
                        # rowsums -> reciprocals
                        rcps = []
                        for hh in range(2):
                            prs = ps_misc.tile([P, TC], F32, tag="ps_misc")
                            nc.tensor.matmul(prs[0:1, :], on_b[:],
                                             rs[:, hh * TC:(hh + 1) * TC],
                                             start=True, stop=True)
                            rcp = work.tile([1, TC], F32R, tag="rcp")
                            with nc.allow_low_precision(reason="f32r reciprocal is plenty for softmax denom"):
                                nc.vector.reciprocal(rcp[:], prs[0:1, :])
                            rcps.append(rcp)
                        pR = ps_misc.tile([P, TC], F32, tag="ps_misc")
                        nc.tensor.matmul(pR[:], sel_sb[:, 0:P], rcps[0][:],
                                         start=True, stop=False)
                        nc.tensor.matmul(pR[:], sel_sb[:, P:2 * P], rcps[1][:],
                                         start=False, stop=True)
                        R_sb = work.tile([P, TC], F32R, tag="R")
                        nc.vector.tensor_copy(R_sb[:], pR[:])
                        nc.vector.tensor_tensor(
                            ctx_sb[:, pr, t0:t0 + TC], pctx[:], R_sb[:],
                            mybir.AluOpType.mult)

            # ---- phase 3: output projection ----
            with nc.named_scope("outproj"):
                for tt in range(NST):
                    for ec2 in range(2):
                        pso = ps_misc.tile([P, TC], F32, tag="ps_misc")
                        for pr in range(2):
                            nc.tensor.matmul(
                                pso[:], ctx_sb[:, pr, tt * P:(tt + 1) * P],
                                wo_sb[:, pr, ec2 * TC:(ec2 + 1) * TC],
                                start=(pr == 0), stop=False)
                        nc.tensor.matmul(pso[:], on_r[:1, :P],
                                         bo_sb[:, ec2 * TC:(ec2 + 1) * TC],
                                         start=False, stop=True)
                        o_sb = work.tile([P, TC], F32, tag="o")
                        nc.vector.tensor_copy(o_sb[:], pso[:])
                        nc.sync.dma_start(
                            out[tt * P:(tt + 1) * P, ec2 * TC:(ec2 + 1) * TC],
                            o_sb[:])
    nc.compile()
    return nc


_NC = None


def _get_nc():
    global _NC
    if _NC is None:
        _NC = build_kernel()
    return _NC


def make_in_maps(hidden_states, Wq, bq, Wk, bk, Wv, bv, Wo, bo, gate):
    f = np.float32
    hidden_states = np.asarray(hidden_states, f)
    Wq, bq = np.asarray(Wq, f), np.asarray(bq, f)
    Wk, bk = np.asarray(Wk, f), np.asarray(bk, f)
    Wv, bv = np.asarray(Wv, f), np.asarray(bv, f)
    Wo, bo = np.asarray(Wo, f), np.asarray(bo, f)
    gate = np.asarray(gate, f)

    hT_b = [np.ascontiguousarray(hidden_states[b].T).reshape(NEC, P, T)
            for b in range(B)]
    ones_r = np.ones((1, TC), f)
    ones_b = np.ones((P, 1), ml_dtypes.bfloat16)
    sel_np = np.zeros((1, 2 * P), f)
    sel_np[0, 0:64] = 1.0      # head-A rows of R
    sel_np[0, P + 64:2 * P] = 1.0  # head-B rows of R

    in_maps = []
    for core in range(NCORES):
        b, hg = divmod(core, 4)
        hs = [4 * hg + i for i in range(4)]
        # [2, 128, NEC, 128]: per pair, (e_in, e_chunk, d-packed)
        def pack_qk(W):
            outw = np.empty((2, P, NEC, P), f)
            for pr in range(2):
                pair = np.concatenate(
                    [W[hs[2 * pr]], W[hs[2 * pr + 1]]], axis=1)  # [E, 128]
                outw[pr] = pair.reshape(NEC, P, P).transpose(1, 0, 2)
            return outw
        wv_np = np.concatenate([Wv[h] for h in hs], axis=1)  # [E, 256]
        wv_np = wv_np.reshape(NEC, P, 256).transpose(1, 0, 2)
        wo_np = np.empty((2, P, E), f)
        bq_np = np.empty((2, 1, P), f)
        bk_np = np.empty((2, 1, P), f)
        for pr in range(2):
            h0, h1 = hs[2 * pr], hs[2 * pr + 1]
            wo_np[pr] = np.concatenate(
                [gate[h0] * Wo[h0], gate[h1] * Wo[h1]], axis=0)  # [128, E]
            bq_np[pr, 0] = np.concatenate([bq[h0], bq[h1]])
            bk_np[pr, 0] = np.concatenate([bk[h0], bk[h1]])
        bv_np = np.concatenate([bv[h] for h in hs])[None, :]  # [1, 256]
        bo_np = sum(gate[h] * bo[h] for h in hs)[None, :]     # [1, E]
        in_maps.append(dict(
            hT=np.ascontiguousarray(hT_b[b]),
            wq=np.ascontiguousarray(pack_qk(Wq)),
            wk=np.ascontiguousarray(pack_qk(Wk)),
            wv=np.ascontiguousarray(wv_np),
            wo=np.ascontiguousarray(wo_np),
            bq=bq_np, bk=bk_np,
            bv=np.ascontiguousarray(bv_np),
            bo=np.ascontiguousarray(bo_np),
            ones_r=ones_r, ones_b=ones_b, sel=sel_np,
        ))
    return in_maps


def kernel(hidden_states, Wq, bq, Wk, bk, Wv, bv, Wo, bo, gate, _trace=False,
           **run_kwargs):
    nc = _get_nc()
    in_maps = make_in_maps(hidden_states, Wq, bq, Wk, bk, Wv, bv, Wo, bo, gate)
    res = bass_utils.run_bass_kernel_spmd(
        nc, in_maps, core_ids=list(range(NCORES)), trace=_trace, **run_kwargs)
    outs = [r["out"] for r in res.results]
    full = np.stack([
        outs[0] + outs[1] + outs[2] + outs[3],
        outs[4] + outs[5] + outs[6] + outs[7],
    ]).astype(np.float32)
    kernel.last_result = res
    return full



# revision 6
# speedup vs baseline: 1.7188x; 1.7188x over previous
"""Gated multi-head self-attention on 8 Trainium2 NeuronCores.

Sharding: batch (B=2) x head-groups (4 groups of 4 heads) -> 8 cores.
Each core computes, for its batch b and its 4 heads:
    partial_out[t, e] = sum_h gate[h] * softmax(Q_h K_h^T / 8) V_h Wo_h
The host sums the 4 head-group partials per batch, adds the constant
term sum_h gate_h*(bo_h + bv_h Wo_h) (bv/bo commute past the softmax
normalization), and stacks the two batches.

v2 design (ACT exp is the critical path: 8 groups x 16 x [128,1024]
exps ~= 141us/core):
  - all matmul inputs bf16 (halves DMA, enables FWL weight loads);
    scores themselves accumulate in fp32 PSUM so softmax is accurate
  - no K=1 bias matmuls: bq/bk added during the DVE eviction of Q/K
    (per-partition scalar add), bv/bo folded into a host-side constant
  - rowsum: DVE adds ex tiles into 4 partial sums, PE accumulates the
    partials via [128,2]-ones stationary matmuls -> [2,512] PSUM row
    per head, reciprocal_approx_fast, one sel2 broadcast matmul
  - attention groups pr-outer; scores/exp stream ahead, PV lags by 2;
    V-proj, remaining Q/K projections and outproj are emitted after the
    group that needs them next, so the Tile scheduler (priority =
    emission order) runs them in PE slack under the ACT-bound groups
"""

import numpy as np
import ml_dtypes
from contextlib import ExitStack

import concourse.bass as bass
import concourse.tile as tile
from concourse import bacc, mybir
from concourse import bass_utils

E, H, D = 1024, 16, 64
B, T = 2, 2048
NCORES = 8
P = 128
TC = 512          # t-chunk (PSUM bank = 512 fp32)
NTC = T // TC     # 4 t-chunks
NST = T // P      # 16 s-tiles
NEC = E // P      # 8 e-chunks

F32 = mybir.dt.float32
F32R = mybir.dt.float32r
BF16 = mybir.dt.bfloat16
ADD = mybir.AluOpType.add
MULT = mybir.AluOpType.mult


def build_kernel():
    nc = bacc.Bacc("TRN2", target_bir_lowering=False, debug=False,
                   num_devices=NCORES)
    hT = nc.dram_tensor("hT", [NEC, P, T], BF16, kind="ExternalInput").ap()
    wq = nc.dram_tensor("wq", [P, 2, NEC, P], BF16, kind="ExternalInput").ap()
    wk = nc.dram_tensor("wk", [P, 2, NEC, P], BF16, kind="ExternalInput").ap()
    wv = nc.dram_tensor("wv", [P, NEC, 256], BF16, kind="ExternalInput").ap()
    wo = nc.dram_tensor("wo", [P, 2, E], BF16, kind="ExternalInput").ap()
    bq = nc.dram_tensor("bq", [P, 2], F32, kind="ExternalInput").ap()
    bk = nc.dram_tensor("bk", [P, 2], F32, kind="ExternalInput").ap()
    on2 = nc.dram_tensor("on2", [P, 4], BF16, kind="ExternalInput").ap()
    sel2 = nc.dram_tensor("sel2", [2, P], BF16, kind="ExternalInput").ap()
    out = nc.dram_tensor("out", [T, E], F32, kind="ExternalOutput").ap()

    with tile.TileContext(nc) as tc:
        with ExitStack() as ctx:
            persist = ctx.enter_context(tc.tile_pool(name="persist", bufs=1))
            expool = ctx.enter_context(tc.tile_pool(name="expool", bufs=8))
            rspool = ctx.enter_context(tc.tile_pool(name="rspool", bufs=2))
            rcpool = ctx.enter_context(tc.tile_pool(name="rcpool", bufs=2))
            work = ctx.enter_context(tc.tile_pool(name="work", bufs=4))
            ps_s = ctx.enter_context(tc.tile_pool(name="ps_s", bufs=2, space="PSUM"))
            ps_ctx = ctx.enter_context(tc.tile_pool(name="ps_ctx", bufs=2, space="PSUM"))
            ps_misc = ctx.enter_context(tc.tile_pool(name="ps_misc", bufs=2, space="PSUM"))

            # ---- persistent SBUF tensors ----
            hT_sb = persist.tile([P, NEC, T], BF16, tag="hT")
            wq_sb = persist.tile([P, 2, NEC, P], BF16, tag="wq")
            wk_sb = persist.tile([P, 2, NEC, P], BF16, tag="wk")
            wv_sb = persist.tile([P, NEC, 256], BF16, tag="wv")
            wo_sb = persist.tile([P, 2, E], BF16, tag="wo")
            bq_sb = persist.tile([P, 2], F32, tag="bq")
            bk_sb = persist.tile([P, 2], F32, tag="bk")
            on2_sb = persist.tile([P, 4], BF16, tag="on2")
            sel2_sb = persist.tile([2, P], BF16, tag="sel2")
            QT_sb = persist.tile([P, 2, T], BF16, tag="QT")
            KT_sb = persist.tile([P, 2, T], BF16, tag="KT")
            V_sb = persist.tile([P, NST, 256], BF16, tag="V")
            ctx_sb = persist.tile([P, 2, T], BF16, tag="ctx")

            with nc.named_scope("load"):
                nc.sync.dma_start(wk_sb[:], wk)
                nc.sync.dma_start(wq_sb[:], wq)
                nc.sync.dma_start(bq_sb[:], bq)
                nc.sync.dma_start(bk_sb[:], bk)
                nc.sync.dma_start(on2_sb[:], on2)
                nc.sync.dma_start(sel2_sb[:], sel2)
                for ec in range(NEC):
                    nc.sync.dma_start(hT_sb[:, ec, :], hT[ec])
                nc.sync.dma_start(wv_sb[:], wv)
                nc.sync.dma_start(wo_sb[:], wo)

            def proj_qk(w_sb, b_sb, dst, pr, tch):
                """One [128, TC] chunk of the Q or K projection (+bias)."""
                ps = ps_misc.tile([P, TC], F32, tag="ps_misc")
                for ec in range(NEC):
                    nc.tensor.matmul(
                        ps[:], w_sb[:, pr, ec, :],
                        hT_sb[:, ec, tch * TC:(tch + 1) * TC],
                        start=(ec == 0), stop=(ec == NEC - 1))
                nc.vector.tensor_scalar(
                    dst[:, pr, tch * TC:(tch + 1) * TC], ps[:],
                    b_sb[:, pr:pr + 1], None, ADD)

            def proj_v(st):
                """V rows for s-tile st: [128 t, 256 d] -> V_sb bf16."""
                ps = ps_misc.tile([P, TC], F32, tag="ps_misc")
                psv = ps[:, :256]
                for ec in range(NEC):
                    nc.tensor.matmul(
                        psv, hT_sb[:, ec, st * P:(st + 1) * P],
                        wv_sb[:, ec, :], start=(ec == 0), stop=(ec == NEC - 1))
                nc.vector.tensor_copy(V_sb[:, st, :], psv)

            def outproj(tt):
                """Output projection for t-tile tt (128 t) -> HBM."""
                for ec2 in range(2):
                    pso = ps_misc.tile([P, TC], F32, tag="ps_misc")
                    for pr in range(2):
                        nc.tensor.matmul(
                            pso[:], ctx_sb[:, pr, tt * P:(tt + 1) * P],
                            wo_sb[:, pr, ec2 * TC:(ec2 + 1) * TC],
                            start=(pr == 0), stop=(pr == 1))
                    o_sb = work.tile([P, TC], F32, tag="o")
                    nc.vector.tensor_copy(o_sb[:], pso[:])
                    nc.sync.dma_start(
                        out[tt * P:(tt + 1) * P, ec2 * TC:(ec2 + 1) * TC],
                        o_sb[:])

            def pv(pctx, ex, st, pr):
                c0 = pr * P
                nc.tensor.matmul(
                    pctx[0:64, :], V_sb[:, st, c0:c0 + 64], ex[:, :TC],
                    start=(st == 0), stop=(st == NST - 1),
                    tile_position=(0, 0), skip_group_check=True)
                nc.tensor.matmul(
                    pctx[64:P, :], V_sb[:, st, c0 + 64:c0 + P], ex[:, TC:],
                    start=(st == 0), stop=(st == NST - 1),
                    tile_position=(0, 64), skip_group_check=True)

            # all projections, in semantic order (producers precede the
            # attention consumers).  The attention stream below is wrapped
            # in high_priority so the scheduler runs these only in PE slack.
            with nc.named_scope("qkv"):
                for tch in range(NTC):
                    proj_qk(wk_sb, bk_sb, KT_sb, 0, tch)
                proj_qk(wq_sb, bq_sb, QT_sb, 0, 0)
                for st in range(NST):
                    proj_v(st)
                for tch in range(1, NTC):
                    proj_qk(wq_sb, bq_sb, QT_sb, 0, tch)
                for tch in range(NTC):
                    proj_qk(wk_sb, bk_sb, KT_sb, 1, tch)
                for tch in range(NTC):
                    proj_qk(wq_sb, bq_sb, QT_sb, 1, tch)

            # outproj t-tiles become legal after the pr=1 group of their tch
            outproj_sched = {4: [0, 1], 5: [2, 3, 4, 5], 6: [6, 7, 8, 9],
                             7: list(range(10, NST))}

            # ---- attention: 8 groups, pr-outer ----
            with nc.named_scope("attn"):
                groups = [(tch, pr) for pr in range(2) for tch in range(NTC)]
                for gi, (tch, pr) in enumerate(groups):
                    hp = tc.high_priority(offset=1_000_000)
                    hp.__enter__()
                    t0 = tch * TC
                    pctx = ps_ctx.tile([P, TC], F32, tag="ps_ctx")
                    # 4 partial row-sums, chains of 4 ex tiles each
                    rs4 = []
                    for i in range(4):
                        rs_i = rspool.tile([P, 2 * TC], BF16, tag=f"rs{i}",
                                           name=f"rs{i}_{gi}")
                        rs4.append(rs_i)
                    exs = [None] * NST
                    for st in range(NST):
                        s0 = st * P
                        pss = ps_s.tile([P, 2 * TC], F32, tag="ps_s")
                        nc.tensor.matmul(
                            pss[:, :TC], KT_sb[0:64, pr, s0:s0 + P],
                            QT_sb[0:64, pr, t0:t0 + TC],
                            start=True, stop=True, tile_position=(0, 0))
                        nc.tensor.matmul(
                            pss[:, TC:], KT_sb[64:P, pr, s0:s0 + P],
                            QT_sb[64:P, pr, t0:t0 + TC],
                            start=True, stop=True, tile_position=(64, 0))
                        ex = expool.tile([P, 2 * TC], BF16, tag="expT")
                        exs[st] = ex
                        nc.scalar.activation(
                            ex[:], pss[:],
                            mybir.ActivationFunctionType.Exp, scale=0.125)
                        ci, cj = divmod(st, 4)
                        if cj == 1:
                            nc.vector.tensor_tensor(
                                rs4[ci][:], exs[st - 1][:], ex[:], ADD)
                        elif cj > 1:
                            nc.vector.tensor_tensor(
                                rs4[ci][:], rs4[ci][:], ex[:], ADD)
                        if st >= 2:
                            pv(pctx, exs[st - 2], st - 2, pr)
                    pv(pctx, exs[NST - 2], NST - 2, pr)
                    pv(pctx, exs[NST - 1], NST - 1, pr)

                    # normalize tail: rowsum partials -> [2,TC] -> 1/x ->
                    # broadcast to [128,TC] -> scale ctx
                    prs = ps_misc.tile([P, TC], F32, tag="ps_misc")
                    for i in range(4):
                        nc.tensor.matmul(prs[0:2, :], on2_sb[:, 0:2],
                                         rs4[i][:, :TC],
                                         start=(i == 0), stop=False)
                        nc.tensor.matmul(prs[0:2, :], on2_sb[:, 2:4],
                                         rs4[i][:, TC:],
                                         start=False, stop=(i == 3))
                    rcp = rcpool.tile([2, TC], F32, tag="rcp")
                    nc.vector.reciprocal_approx_fast(rcp[:], prs[0:2, :])
                    rcp_bf = rcpool.tile([2, TC], BF16, tag="rcpb")
                    nc.vector.tensor_copy(rcp_bf[:], rcp[:])
                    pR = ps_misc.tile([P, TC], F32, tag="ps_misc")
                    nc.tensor.matmul(
                        pR[:], sel2_sb[:], rcp_bf[:], start=True, stop=True)
                    R_sb = work.tile([P, TC], F32, tag="R")
                    nc.vector.tensor_copy(R_sb[:], pR[:])
                    nc.vector.tensor_tensor(
                        ctx_sb[:, pr, t0:t0 + TC], pctx[:], R_sb[:], MULT)
                    hp.__exit__(None, None, None)

                    for tt in outproj_sched.get(gi, []):
                        outproj(tt)
    nc.compile()
    return nc


_NC = None


def _get_nc():
    global _NC
    if _NC is None:
        _NC = build_kernel()
    return _NC


def make_in_maps(hidden_states, Wq, bq, Wk, bk, Wv, bv, Wo, bo, gate):
    f = np.float32
    b16 = ml_dtypes.bfloat16
    hidden_states = np.asarray(hidden_states, f)
    Wq, bq = np.asarray(Wq, f), np.asarray(bq, f)
    Wk, bk = np.asarray(Wk, f), np.asarray(bk, f)
    Wv, bv = np.asarray(Wv, f), np.asarray(bv, f)
    Wo, bo = np.asarray(Wo, f), np.asarray(bo, f)
    gate = np.asarray(gate, f)

    hT_b = [np.ascontiguousarray(hidden_states[b].T)
            .reshape(NEC, P, T).astype(b16) for b in range(B)]
    on2_np = np.zeros((P, 4), b16)
    on2_np[:, 0] = 1.0   # head-A rowsum -> psum row 0
    on2_np[:, 3] = 1.0   # head-B rowsum -> psum row 1
    sel2_np = np.zeros((2, P), b16)
    sel2_np[0, 0:64] = 1.0
    sel2_np[1, 64:P] = 1.0

    in_maps = []
    consts = []
    for core in range(NCORES):
        b, hg = divmod(core, 4)
        hs = [4 * hg + i for i in range(4)]

        def pack_qk(W):
            outw = np.empty((P, 2, NEC, P), f)
            for pr in range(2):
                pair = np.concatenate(
                    [W[hs[2 * pr]], W[hs[2 * pr + 1]]], axis=1)  # [E, 128]
                outw[:, pr] = pair.reshape(NEC, P, P).transpose(1, 0, 2)
            return outw.astype(b16)

        wv_np = np.concatenate([Wv[h] for h in hs], axis=1)  # [E, 256]
        wv_np = wv_np.reshape(NEC, P, 256).transpose(1, 0, 2).astype(b16)
        wo_np = np.empty((2, P, E), f)
        bq_np = np.empty((P, 2), f)
        bk_np = np.empty((P, 2), f)
        for pr in range(2):
            h0, h1 = hs[2 * pr], hs[2 * pr + 1]
            wo_np[pr] = np.concatenate(
                [gate[h0] * Wo[h0], gate[h1] * Wo[h1]], axis=0)  # [128, E]
            bq_np[:, pr] = np.concatenate([bq[h0], bq[h1]])
            bk_np[:, pr] = np.concatenate([bk[h0], bk[h1]])
        # constant term: sum_h gate_h * (bo_h + bv_h @ Wo_h)   [E]
        cst = sum(gate[h] * (bo[h] + bv[h] @ Wo[h]) for h in hs)
        consts.append(np.asarray(cst, f))
        in_maps.append(dict(
            hT=np.ascontiguousarray(hT_b[b]),
            wq=np.ascontiguousarray(pack_qk(Wq)),
            wk=np.ascontiguousarray(pack_qk(Wk)),
            wv=np.ascontiguousarray(wv_np),
            wo=np.ascontiguousarray(wo_np.transpose(1, 0, 2).astype(b16)),
            bq=bq_np, bk=bk_np,
            on2=on2_np, sel2=sel2_np,
        ))
    return in_maps, consts


def kernel(hidden_states, Wq, bq, Wk, bk, Wv, bv, Wo, bo, gate, _trace=False,
           **run_kwargs):
    nc = _get_nc()
    in_maps, consts = make_in_maps(
        hidden_states, Wq, bq, Wk, bk, Wv, bv, Wo, bo, gate)
    res = bass_utils.run_bass_kernel_spmd(
        nc, in_maps, core_ids=list(range(NCORES)), trace=_trace, **run_kwargs)
    outs = [r["out"] for r in res.results]
    full = np.stack([
        outs[0] + outs[1] + outs[2] + outs[3]
        + (consts[0] + consts[1] + consts[2] + consts[3])[None, :],
        outs[4] + outs[5] + outs[6] + outs[7]
        + (consts[4] + consts[5] + consts[6] + consts[7])[None, :],
    ]).astype(np.float32)
    kernel.last_result = res
    return full


# revision 7
# speedup vs baseline: 1.7489x; 1.0175x over previous
"""Gated multi-head self-attention on 8 Trainium2 NeuronCores.

Sharding: batch (B=2) x head-groups (4 groups of 4 heads) -> 8 cores.
Each core computes, for its batch b and its 4 heads:
    partial_out[t, e] = sum_h gate[h] * softmax(Q_h K_h^T / 8) V_h Wo_h
The host sums the 4 head-group partials per batch, adds the constant
term sum_h gate_h*(bo_h + bv_h Wo_h) (bv/bo commute past the softmax
normalization), and stacks the two batches.

v2 design (ACT exp is the critical path: 8 groups x 16 x [128,1024]
exps ~= 141us/core):
  - all matmul inputs bf16 (halves DMA, enables FWL weight loads);
    scores themselves accumulate in fp32 PSUM so softmax is accurate
  - no K=1 bias matmuls: bq/bk added during the DVE eviction of Q/K
    (per-partition scalar add), bv/bo folded into a host-side constant
  - rowsum: DVE adds ex tiles into 4 partial sums, PE accumulates the
    partials via [128,2]-ones stationary matmuls -> [2,512] PSUM row
    per head, reciprocal_approx_fast, one sel2 broadcast matmul
  - attention groups pr-outer; scores/exp stream ahead, PV lags by 2;
    V-proj, remaining Q/K projections and outproj are emitted after the
    group that needs them next, so the Tile scheduler (priority =
    emission order) runs them in PE slack under the ACT-bound groups
"""

import numpy as np
import ml_dtypes
from contextlib import ExitStack

import concourse.bass as bass
import concourse.tile as tile
from concourse import bacc, mybir
from concourse import bass_utils

E, H, D = 1024, 16, 64
B, T = 2, 2048
NCORES = 8
P = 128
TC = 512          # t-chunk (PSUM bank = 512 fp32)
NTC = T // TC     # 4 t-chunks
NST = T // P      # 16 s-tiles
NEC = E // P      # 8 e-chunks

F32 = mybir.dt.float32
F32R = mybir.dt.float32r
BF16 = mybir.dt.bfloat16
ADD = mybir.AluOpType.add
MULT = mybir.AluOpType.mult


def build_kernel():
    nc = bacc.Bacc("TRN2", target_bir_lowering=False, debug=False,
                   num_devices=NCORES)
    hT = nc.dram_tensor("hT", [NEC, P, T], BF16, kind="ExternalInput").ap()
    wq = nc.dram_tensor("wq", [P, 2, NEC, P], BF16, kind="ExternalInput").ap()
    wk = nc.dram_tensor("wk", [P, 2, NEC, P], BF16, kind="ExternalInput").ap()
    wv = nc.dram_tensor("wv", [P, NEC, 256], BF16, kind="ExternalInput").ap()
    wo = nc.dram_tensor("wo", [P, 2, E], BF16, kind="ExternalInput").ap()
    bq = nc.dram_tensor("bq", [P, 2], F32, kind="ExternalInput").ap()
    bk = nc.dram_tensor("bk", [P, 2], F32, kind="ExternalInput").ap()
    on2 = nc.dram_tensor("on2", [P, 4], BF16, kind="ExternalInput").ap()
    sel2 = nc.dram_tensor("sel2", [2, P], BF16, kind="ExternalInput").ap()
    out = nc.dram_tensor("out", [T, E], F32, kind="ExternalOutput").ap()

    with tile.TileContext(nc) as tc:
        with ExitStack() as ctx:
            persist = ctx.enter_context(tc.tile_pool(name="persist", bufs=1))
            expool = ctx.enter_context(tc.tile_pool(name="expool", bufs=20))
            rspool = ctx.enter_context(tc.tile_pool(name="rspool", bufs=2))
            rcpool = ctx.enter_context(tc.tile_pool(name="rcpool", bufs=2))
            work = ctx.enter_context(tc.tile_pool(name="work", bufs=4))
            ps_s = ctx.enter_context(tc.tile_pool(name="ps_s", bufs=2, space="PSUM"))
            ps_ctx = ctx.enter_context(tc.tile_pool(name="ps_ctx", bufs=2, space="PSUM"))
            ps_misc = ctx.enter_context(tc.tile_pool(name="ps_misc", bufs=2, space="PSUM"))

            # ---- persistent SBUF tensors ----
            hT_sb = persist.tile([P, NEC, T], BF16, tag="hT")
            wq_sb = persist.tile([P, 2, NEC, P], BF16, tag="wq")
            wk_sb = persist.tile([P, 2, NEC, P], BF16, tag="wk")
            wv_sb = persist.tile([P, NEC, 256], BF16, tag="wv")
            wo_sb = persist.tile([P, 2, E], BF16, tag="wo")
            bq_sb = persist.tile([P, 2], F32, tag="bq")
            bk_sb = persist.tile([P, 2], F32, tag="bk")
            on2_sb = persist.tile([P, 4], BF16, tag="on2")
            sel2_sb = persist.tile([2, P], BF16, tag="sel2")
            QT_sb = persist.tile([P, 2, T], BF16, tag="QT")
            KT_sb = persist.tile([P, 2, T], BF16, tag="KT")
            V_sb = persist.tile([P, NST, 256], BF16, tag="V")
            ctx_sb = persist.tile([P, 2, T], BF16, tag="ctx")

            with nc.named_scope("load"):
                nc.sync.dma_start(wk_sb[:], wk)
                nc.sync.dma_start(wq_sb[:], wq)
                nc.sync.dma_start(bq_sb[:], bq)
                nc.sync.dma_start(bk_sb[:], bk)
                nc.sync.dma_start(on2_sb[:], on2)
                nc.sync.dma_start(sel2_sb[:], sel2)
                for ec in range(NEC):
                    nc.sync.dma_start(hT_sb[:, ec, :], hT[ec])
                nc.sync.dma_start(wv_sb[:], wv)
                nc.sync.dma_start(wo_sb[:], wo)

            def proj_qk(w_sb, b_sb, dst, pr, tch):
                """One [128, TC] chunk of the Q or K projection (+bias)."""
                ps = ps_misc.tile([P, TC], F32, tag="ps_misc")
                for ec in range(NEC):
                    nc.tensor.matmul(
                        ps[:], w_sb[:, pr, ec, :],
                        hT_sb[:, ec, tch * TC:(tch + 1) * TC],
                        start=(ec == 0), stop=(ec == NEC - 1))
                nc.vector.tensor_scalar(
                    dst[:, pr, tch * TC:(tch + 1) * TC], ps[:],
                    b_sb[:, pr:pr + 1], None, ADD)

            def proj_v(st):
                """V rows for s-tile st: [128 t, 256 d] -> V_sb bf16."""
                ps = ps_misc.tile([P, TC], F32, tag="ps_misc")
                psv = ps[:, :256]
                for ec in range(NEC):
                    nc.tensor.matmul(
                        psv, hT_sb[:, ec, st * P:(st + 1) * P],
                        wv_sb[:, ec, :], start=(ec == 0), stop=(ec == NEC - 1))
                nc.vector.tensor_copy(V_sb[:, st, :], psv)

            def outproj(tt):
                """Output projection for t-tile tt (128 t) -> HBM."""
                for ec2 in range(2):
                    pso = ps_misc.tile([P, TC], F32, tag="ps_misc")
                    for pr in range(2):
                        nc.tensor.matmul(
                            pso[:], ctx_sb[:, pr, tt * P:(tt + 1) * P],
                            wo_sb[:, pr, ec2 * TC:(ec2 + 1) * TC],
                            start=(pr == 0), stop=(pr == 1))
                    o_sb = work.tile([P, TC], F32, tag="o")
                    nc.vector.tensor_copy(o_sb[:], pso[:])
                    nc.sync.dma_start(
                        out[tt * P:(tt + 1) * P, ec2 * TC:(ec2 + 1) * TC],
                        o_sb[:])

            def pv(pctx, ex, st, pr):
                c0 = pr * P
                nc.tensor.matmul(
                    pctx[0:64, :], V_sb[:, st, c0:c0 + 64], ex[:, :TC],
                    start=(st == 0), stop=(st == NST - 1),
                    tile_position=(0, 0), skip_group_check=True)
                nc.tensor.matmul(
                    pctx[64:P, :], V_sb[:, st, c0 + 64:c0 + P], ex[:, TC:],
                    start=(st == 0), stop=(st == NST - 1),
                    tile_position=(0, 64), skip_group_check=True)

            with nc.named_scope("qkv"):
                for tch in range(NTC):
                    proj_qk(wk_sb, bk_sb, KT_sb, 0, tch)
                proj_qk(wq_sb, bq_sb, QT_sb, 0, 0)

            # work quanta interleaved one-per-iteration into the attention
            # groups (program order = semantic order AND scheduler priority):
            # every quantum is emitted before its first consumer.
            extras = {
                0: [lambda: proj_qk(wq_sb, bq_sb, QT_sb, 0, 1)]
                   + [(lambda st=st: proj_v(st)) for st in range(NST)],
                1: [lambda: proj_qk(wq_sb, bq_sb, QT_sb, 0, 2)]
                   + [(lambda t=t: proj_qk(wk_sb, bk_sb, KT_sb, 1, t))
                      for t in range(NTC)],
                2: [lambda: proj_qk(wq_sb, bq_sb, QT_sb, 0, 3),
                    lambda: proj_qk(wq_sb, bq_sb, QT_sb, 1, 0),
                    lambda: proj_qk(wq_sb, bq_sb, QT_sb, 1, 1)],
                3: [lambda: proj_qk(wq_sb, bq_sb, QT_sb, 1, 2),
                    lambda: proj_qk(wq_sb, bq_sb, QT_sb, 1, 3)],
                5: [lambda tt=tt: outproj(tt) for tt in range(0, 4)],
                6: [lambda tt=tt: outproj(tt) for tt in range(4, 8)],
                7: [lambda tt=tt: outproj(tt) for tt in range(8, 12)],
            }

            # ---- attention: 8 groups, pr-outer ----
            # Group 0's PV matmuls and tail are deferred into group 1 so
            # V-proj (interleaved through group 0) has a full group of
            # margin; later groups run PV with a 2-iteration lag.
            def tail(gi, tch, pr, pctx, rs4):
                t0 = tch * TC
                prs = ps_misc.tile([P, TC], F32, tag="ps_misc",
                                   name=f"prs_{gi}")
                for i in range(4):
                    nc.tensor.matmul(prs[0:2, :], on2_sb[:, 0:2],
                                     rs4[i][:, :TC],
                                     start=(i == 0), stop=False)
                    nc.tensor.matmul(prs[0:2, :], on2_sb[:, 2:4],
                                     rs4[i][:, TC:],
                                     start=False, stop=(i == 3))
                rcp = rcpool.tile([2, TC], F32, tag="rcp")
                nc.vector.reciprocal_approx_fast(rcp[:], prs[0:2, :])
                rcp_bf = rcpool.tile([2, TC], BF16, tag="rcpb")
                nc.vector.tensor_copy(rcp_bf[:], rcp[:])
                pR = ps_misc.tile([P, TC], F32, tag="ps_misc",
                                  name=f"pR_{gi}")
                nc.tensor.matmul(
                    pR[:], sel2_sb[:], rcp_bf[:], start=True, stop=True)
                R_sb = work.tile([P, TC], F32, tag="R", name=f"R_{gi}")
                nc.vector.tensor_copy(R_sb[:], pR[:])
                nc.vector.tensor_tensor(
                    ctx_sb[:, pr, t0:t0 + TC], pctx[:], R_sb[:], MULT)

            with nc.named_scope("attn"):
                groups = [(tch, pr) for pr in range(2) for tch in range(NTC)]
                prev = None   # deferred (pctx, exs, pr, rs4) of group 0
                for gi, (tch, pr) in enumerate(groups):
                    t0 = tch * TC
                    ex_quota = extras.get(gi, [])
                    qi = 0
                    pctx = ps_ctx.tile([P, TC], F32, tag="ps_ctx")
                    rs4 = []
                    for i in range(4):
                        rs_i = rspool.tile([P, 2 * TC], BF16, tag=f"rs{i}",
                                           name=f"rs{i}_{gi}")
                        rs4.append(rs_i)
                    exs = [None] * NST
                    for st in range(NST):
                        s0 = st * P
                        pss = ps_s.tile([P, 2 * TC], F32, tag="ps_s")
                        nc.tensor.matmul(
                            pss[:, :TC], KT_sb[0:64, pr, s0:s0 + P],
                            QT_sb[0:64, pr, t0:t0 + TC],
                            start=True, stop=True, tile_position=(0, 0))
                        nc.tensor.matmul(
                            pss[:, TC:], KT_sb[64:P, pr, s0:s0 + P],
                            QT_sb[64:P, pr, t0:t0 + TC],
                            start=True, stop=True, tile_position=(64, 0))
                        ex = expool.tile([P, 2 * TC], BF16, tag="expT")
                        exs[st] = ex
                        nc.scalar.activation(
                            ex[:], pss[:],
                            mybir.ActivationFunctionType.Exp, scale=0.125)
                        ci, cj = divmod(st, 4)
                        if cj == 1:
                            nc.vector.tensor_tensor(
                                rs4[ci][:], exs[st - 1][:], ex[:], ADD)
                        elif cj > 1:
                            nc.vector.tensor_tensor(
                                rs4[ci][:], rs4[ci][:], ex[:], ADD)
                        if qi < len(ex_quota):
                            ex_quota[qi]()
                            qi += 1
                        if prev is not None:
                            # group 0's PV, one pair per iteration of group 1
                            pv(prev[0], prev[1][st], st, prev[2])
                            if st == NST - 1:
                                tail(gi - 1, 0, prev[2], prev[0], prev[3])
                        if gi >= 1 and st >= 2:
                            pv(pctx, exs[st - 2], st - 2, pr)
                    while qi < len(ex_quota):
                        ex_quota[qi]()
                        qi += 1
                    if gi == 0:
                        prev = (pctx, exs, pr, rs4)
                        continue
                    if gi == 1:
                        prev = None
                    pv(pctx, exs[NST - 2], NST - 2, pr)
                    pv(pctx, exs[NST - 1], NST - 1, pr)
                    tail(gi, tch, pr, pctx, rs4)

            with nc.named_scope("outproj"):
                for tt in range(12, NST):
                    outproj(tt)
    nc.compile()
    return nc


_NC = None


def _get_nc():
    global _NC
    if _NC is None:
        _NC = build_kernel()
    return _NC


def make_in_maps(hidden_states, Wq, bq, Wk, bk, Wv, bv, Wo, bo, gate):
    f = np.float32
    b16 = ml_dtypes.bfloat16
    hidden_states = np.asarray(hidden_states, f)
    Wq, bq = np.asarray(Wq, f), np.asarray(bq, f)
    Wk, bk = np.asarray(Wk, f), np.asarray(bk, f)
    Wv, bv = np.asarray(Wv, f), np.asarray(bv, f)
    Wo, bo = np.asarray(Wo, f), np.asarray(bo, f)
    gate = np.asarray(gate, f)

    hT_b = [np.ascontiguousarray(hidden_states[b].T)
            .reshape(NEC, P, T).astype(b16) for b in range(B)]
    on2_np = np.zeros((P, 4), b16)
    on2_np[:, 0] = 1.0   # head-A rowsum -> psum row 0
    on2_np[:, 3] = 1.0   # head-B rowsum -> psum row 1
    sel2_np = np.zeros((2, P), b16)
    sel2_np[0, 0:64] = 1.0
    sel2_np[1, 64:P] = 1.0

    in_maps = []
    consts = []
    for core in range(NCORES):
        b, hg = divmod(core, 4)
        hs = [4 * hg + i for i in range(4)]

        def pack_qk(W):
            outw = np.empty((P, 2, NEC, P), f)
            for pr in range(2):
                pair = np.concatenate(
                    [W[hs[2 * pr]], W[hs[2 * pr + 1]]], axis=1)  # [E, 128]
                outw[:, pr] = pair.reshape(NEC, P, P).transpose(1, 0, 2)
            return outw.astype(b16)

        wv_np = np.concatenate([Wv[h] for h in hs], axis=1)  # [E, 256]
        wv_np = wv_np.reshape(NEC, P, 256).transpose(1, 0, 2).astype(b16)
        wo_np = np.empty((2, P, E), f)
        bq_np = np.empty((P, 2), f)
        bk_np = np.empty((P, 2), f)
        for pr in range(2):
            h0, h1 = hs[2 * pr], hs[2 * pr + 1]
            wo_np[pr] = np.concatenate(
                [gate[h0] * Wo[h0], gate[h1] * Wo[h1]], axis=0)  # [128, E]
            bq_np[:, pr] = np.concatenate([bq[h0], bq[h1]])
            bk_np[:, pr] = np.concatenate([bk[h0], bk[h1]])
        # constant term: sum_h gate_h * (bo_h + bv_h @ Wo_h)   [E]
        cst = sum(gate[h] * (bo[h] + bv[h] @ Wo[h]) for h in hs)
        consts.append(np.asarray(cst, f))
        in_maps.append(dict(
            hT=np.ascontiguousarray(hT_b[b]),
            wq=np.ascontiguousarray(pack_qk(Wq)),
            wk=np.ascontiguousarray(pack_qk(Wk)),
            wv=np.ascontiguousarray(wv_np),
            wo=np.ascontiguousarray(wo_np.transpose(1, 0, 2).astype(b16)),
            bq=bq_np, bk=bk_np,
            on2=on2_np, sel2=sel2_np,
        ))
    return in_maps, consts


def kernel(hidden_states, Wq, bq, Wk, bk, Wv, bv, Wo, bo, gate, _trace=False,
           **run_kwargs):
    nc = _get_nc()
    in_maps, consts = make_in_maps(
        hidden_states, Wq, bq, Wk, bk, Wv, bv, Wo, bo, gate)
    res = bass_utils.run_bass_kernel_spmd(
        nc, in_maps, core_ids=list(range(NCORES)), trace=_trace, **run_kwargs)
    outs = [r["out"] for r in res.results]
    full = np.stack([
        outs[0] + outs[1] + outs[2] + outs[3]
        + (consts[0] + consts[1] + consts[2] + consts[3])[None, :],
        outs[4] + outs[5] + outs[6] + outs[7]
        + (consts[4] + consts[5] + consts[6] + consts[7])[None, :],
    ]).astype(np.float32)
    kernel.last_result = res
    return full


# revision 9
# speedup vs baseline: 1.8198x; 1.0406x over previous
"""Gated multi-head self-attention on 8 Trainium2 NeuronCores.

Sharding: batch (B=2) x head-groups (4 groups of 4 heads) -> 8 cores.
Each core computes, for its batch b and its 4 heads:
    partial_out[t, e] = sum_h gate[h] * softmax(Q_h K_h^T / 8) V_h Wo_h
The host sums the 4 head-group partials per batch, adds the constant
term sum_h gate_h*(bo_h + bv_h Wo_h) (bv/bo commute past the softmax
normalization), and stacks the two batches.

v2 design (ACT exp is the critical path: 8 groups x 16 x [128,1024]
exps ~= 141us/core):
  - all matmul inputs bf16 (halves DMA, enables FWL weight loads);
    scores themselves accumulate in fp32 PSUM so softmax is accurate
  - no K=1 bias matmuls: bq/bk added during the DVE eviction of Q/K
    (per-partition scalar add), bv/bo folded into a host-side constant
  - rowsum: DVE adds ex tiles into 4 partial sums, PE accumulates the
    partials via [128,2]-ones stationary matmuls -> [2,512] PSUM row
    per head, reciprocal_approx_fast, one sel2 broadcast matmul
  - attention groups pr-outer; scores/exp stream ahead, PV lags by 2;
    V-proj, remaining Q/K projections and outproj are emitted after the
    group that needs them next, so the Tile scheduler (priority =
    emission order) runs them in PE slack under the ACT-bound groups
"""

import numpy as np
import ml_dtypes
from contextlib import ExitStack

import concourse.bass as bass
import concourse.tile as tile
from concourse import bacc, mybir
from concourse import bass_utils

E, H, D = 1024, 16, 64
B, T = 2, 2048
NCORES = 8
P = 128
TC = 512          # t-chunk (PSUM bank = 512 fp32)
NTC = T // TC     # 4 t-chunks
NST = T // P      # 16 s-tiles
NEC = E // P      # 8 e-chunks

F32 = mybir.dt.float32
F32R = mybir.dt.float32r
BF16 = mybir.dt.bfloat16
ADD = mybir.AluOpType.add
MULT = mybir.AluOpType.mult


def build_kernel():
    nc = bacc.Bacc("TRN2", target_bir_lowering=False, debug=False,
                   num_devices=NCORES)
    hT = nc.dram_tensor("hT", [NEC, P, T], BF16, kind="ExternalInput").ap()
    wq = nc.dram_tensor("wq", [P, 2, NEC, P], BF16, kind="ExternalInput").ap()
    wk = nc.dram_tensor("wk", [P, 2, NEC, P], BF16, kind="ExternalInput").ap()
    wv = nc.dram_tensor("wv", [P, NEC, 256], BF16, kind="ExternalInput").ap()
    wo = nc.dram_tensor("wo", [P, 2, E], BF16, kind="ExternalInput").ap()
    bq = nc.dram_tensor("bq", [P, 2], F32, kind="ExternalInput").ap()
    bk = nc.dram_tensor("bk", [P, 2], F32, kind="ExternalInput").ap()
    on2 = nc.dram_tensor("on2", [P, 4], BF16, kind="ExternalInput").ap()
    sel2 = nc.dram_tensor("sel2", [2, P], BF16, kind="ExternalInput").ap()
    out = nc.dram_tensor("out", [T, E], F32, kind="ExternalOutput").ap()

    with tile.TileContext(nc) as tc:
        with ExitStack() as ctx:
            persist = ctx.enter_context(tc.tile_pool(name="persist", bufs=1))
            expool = ctx.enter_context(tc.tile_pool(name="expool", bufs=20))
            rspool = ctx.enter_context(tc.tile_pool(name="rspool", bufs=2))
            rcpool = ctx.enter_context(tc.tile_pool(name="rcpool", bufs=2))
            work = ctx.enter_context(tc.tile_pool(name="work", bufs=4))
            ps_s = ctx.enter_context(tc.tile_pool(name="ps_s", bufs=2, space="PSUM"))
            ps_ctx = ctx.enter_context(tc.tile_pool(name="ps_ctx", bufs=2, space="PSUM"))
            ps_misc = ctx.enter_context(tc.tile_pool(name="ps_misc", bufs=2, space="PSUM"))

            # ---- persistent SBUF tensors ----
            hT_sb = persist.tile([P, NEC, T], BF16, tag="hT")
            wq_sb = persist.tile([P, 2, NEC, P], BF16, tag="wq")
            wk_sb = persist.tile([P, 2, NEC, P], BF16, tag="wk")
            wv_sb = persist.tile([P, NEC, 256], BF16, tag="wv")
            wo_sb = persist.tile([P, 2, E], BF16, tag="wo")
            bq_sb = persist.tile([P, 2], F32, tag="bq")
            bk_sb = persist.tile([P, 2], F32, tag="bk")
            on2_sb = persist.tile([P, 4], BF16, tag="on2")
            sel2_sb = persist.tile([2, P], BF16, tag="sel2")
            QT_sb = persist.tile([P, 2, T], BF16, tag="QT")
            KT_sb = persist.tile([P, 2, T], BF16, tag="KT")
            V_sb = persist.tile([P, NST, 256], BF16, tag="V")
            ctx_sb = persist.tile([P, 2, T], BF16, tag="ctx")

            with nc.named_scope("load"):
                # PE warm-up: ~24 dummy matmuls on uninitialized SBUF keep the
                # tensor engine busy during the input DMAs so the HAM clock
                # gate reaches 8/8 (2.4 GHz) before the real work starts.
                for w in range(24):
                    psw = ps_misc.tile([P, TC], F32, tag="ps_misc",
                                       name=f"warm{w}")
                    nc.tensor.matmul(psw[:], KT_sb[0:64, 0, 0:P],
                                     QT_sb[0:64, 0, 0:TC],
                                     start=True, stop=True)
                nc.sync.dma_start(wk_sb[:], wk)
                nc.sync.dma_start(wq_sb[:], wq)
                nc.sync.dma_start(bq_sb[:], bq)
                nc.sync.dma_start(bk_sb[:], bk)
                nc.sync.dma_start(on2_sb[:], on2)
                nc.sync.dma_start(sel2_sb[:], sel2)
                for ec in range(NEC):
                    nc.sync.dma_start(hT_sb[:, ec, :], hT[ec])
                nc.sync.dma_start(wv_sb[:], wv)
                nc.sync.dma_start(wo_sb[:], wo)

            def proj_qk(w_sb, b_sb, dst, pr, tch):
                """One [128, TC] chunk of the Q or K projection (+bias)."""
                ps = ps_misc.tile([P, TC], F32, tag="ps_misc")
                for ec in range(NEC):
                    nc.tensor.matmul(
                        ps[:], w_sb[:, pr, ec, :],
                        hT_sb[:, ec, tch * TC:(tch + 1) * TC],
                        start=(ec == 0), stop=(ec == NEC - 1))
                nc.vector.tensor_scalar(
                    dst[:, pr, tch * TC:(tch + 1) * TC], ps[:],
                    b_sb[:, pr:pr + 1], None, ADD)

            def proj_v(st):
                """V rows for s-tile st: [128 t, 256 d] -> V_sb bf16."""
                ps = ps_misc.tile([P, TC], F32, tag="ps_misc")
                psv = ps[:, :256]
                for ec in range(NEC):
                    nc.tensor.matmul(
                        psv, hT_sb[:, ec, st * P:(st + 1) * P],
                        wv_sb[:, ec, :], start=(ec == 0), stop=(ec == NEC - 1))
                nc.vector.tensor_copy(V_sb[:, st, :], psv)

            def outproj(tt):
                """Output projection for t-tile tt (128 t) -> HBM."""
                for ec2 in range(2):
                    pso = ps_misc.tile([P, TC], F32, tag="ps_misc")
                    for pr in range(2):
                        nc.tensor.matmul(
                            pso[:], ctx_sb[:, pr, tt * P:(tt + 1) * P],
                            wo_sb[:, pr, ec2 * TC:(ec2 + 1) * TC],
                            start=(pr == 0), stop=(pr == 1))
                    o_sb = work.tile([P, TC], F32, tag="o")
                    nc.vector.tensor_copy(o_sb[:], pso[:])
                    nc.sync.dma_start(
                        out[tt * P:(tt + 1) * P, ec2 * TC:(ec2 + 1) * TC],
                        o_sb[:])

            def pv(pctx, ex, st, pr):
                c0 = pr * P
                nc.tensor.matmul(
                    pctx[0:64, :], V_sb[:, st, c0:c0 + 64], ex[:, :TC],
                    start=(st == 0), stop=(st == NST - 1),
                    tile_position=(0, 0), skip_group_check=True)
                nc.tensor.matmul(
                    pctx[64:P, :], V_sb[:, st, c0 + 64:c0 + P], ex[:, TC:],
                    start=(st == 0), stop=(st == NST - 1),
                    tile_position=(0, 64), skip_group_check=True)

            with nc.named_scope("qkv"):
                for tch in range(NTC):
                    proj_qk(wk_sb, bk_sb, KT_sb, 0, tch)
                proj_qk(wq_sb, bq_sb, QT_sb, 0, 0)

            # work quanta interleaved one-per-iteration into the attention
            # groups (program order = semantic order AND scheduler priority):
            # every quantum is emitted before its first consumer.
            extras = {
                0: [lambda: proj_qk(wq_sb, bq_sb, QT_sb, 0, 1)]
                   + [(lambda st=st: proj_v(st)) for st in range(NST)],
                1: [lambda: proj_qk(wq_sb, bq_sb, QT_sb, 0, 2)]
                   + [(lambda t=t: proj_qk(wk_sb, bk_sb, KT_sb, 1, t))
                      for t in range(NTC)],
                2: [lambda: proj_qk(wq_sb, bq_sb, QT_sb, 0, 3),
                    lambda: proj_qk(wq_sb, bq_sb, QT_sb, 1, 0),
                    lambda: proj_qk(wq_sb, bq_sb, QT_sb, 1, 1)],
                3: [lambda: proj_qk(wq_sb, bq_sb, QT_sb, 1, 2),
                    lambda: proj_qk(wq_sb, bq_sb, QT_sb, 1, 3)],
                5: [lambda tt=tt: outproj(tt) for tt in range(0, 4)],
                6: [lambda tt=tt: outproj(tt) for tt in range(4, 8)],
                7: [lambda tt=tt: outproj(tt) for tt in range(8, 12)],
            }

            # ---- attention: 8 groups, pr-outer ----
            # Group 0's PV matmuls and tail are deferred into group 1 so
            # V-proj (interleaved through group 0) has a full group of
            # margin; later groups run PV with a 2-iteration lag.
            def tail(gi, tch, pr, pctx, rs):
                t0 = tch * TC
                prs = ps_misc.tile([P, TC], F32, tag="ps_misc",
                                   name=f"prs_{gi}")
                nc.tensor.matmul(prs[0:2, :], on2_sb[:, 0:2], rs[:, :TC],
                                 start=True, stop=False)
                nc.tensor.matmul(prs[0:2, :], on2_sb[:, 2:4], rs[:, TC:],
                                 start=False, stop=True)
                rcp = rcpool.tile([2, TC], F32, tag="rcp")
                nc.vector.reciprocal_approx_fast(rcp[:], prs[0:2, :])
                rcp_bf = rcpool.tile([2, TC], BF16, tag="rcpb")
                nc.vector.tensor_copy(rcp_bf[:], rcp[:])
                pR = ps_misc.tile([P, TC], F32, tag="ps_misc",
                                  name=f"pR_{gi}")
                nc.tensor.matmul(
                    pR[:], sel2_sb[:], rcp_bf[:], start=True, stop=True)
                R_sb = work.tile([P, TC], F32, tag="R", name=f"R_{gi}")
                nc.vector.tensor_copy(R_sb[:], pR[:])
                nc.vector.tensor_tensor(
                    ctx_sb[:, pr, t0:t0 + TC], pctx[:], R_sb[:], MULT)

            with nc.named_scope("attn"):
                groups = [(tch, pr) for pr in range(2) for tch in range(NTC)]
                prev = None   # deferred (pctx, exs, pr, rs4) of group 0
                for gi, (tch, pr) in enumerate(groups):
                    t0 = tch * TC
                    ex_quota = extras.get(gi, [])
                    qi = 0
                    pctx = ps_ctx.tile([P, TC], F32, tag="ps_ctx")
                    rs = rspool.tile([P, 2 * TC], BF16, tag="rs",
                                     name=f"rs_{gi}")
                    exs = [None] * NST
                    for st in range(NST):
                        s0 = st * P
                        pss = ps_s.tile([P, 2 * TC], F32, tag="ps_s")
                        nc.tensor.matmul(
                            pss[:, :TC], KT_sb[0:64, pr, s0:s0 + P],
                            QT_sb[0:64, pr, t0:t0 + TC],
                            start=True, stop=True, tile_position=(0, 0))
                        nc.tensor.matmul(
                            pss[:, TC:], KT_sb[64:P, pr, s0:s0 + P],
                            QT_sb[64:P, pr, t0:t0 + TC],
                            start=True, stop=True, tile_position=(64, 0))
                        ex = expool.tile([P, 2 * TC], BF16, tag="expT")
                        exs[st] = ex
                        nc.scalar.activation(
                            ex[:], pss[:],
                            mybir.ActivationFunctionType.Exp, scale=0.125)
                        if st == 1:
                            nc.vector.tensor_tensor(
                                rs[:], exs[0][:], ex[:], ADD)
                        elif st > 1:
                            nc.vector.tensor_tensor(rs[:], rs[:], ex[:], ADD)
                        if qi < len(ex_quota):
                            ex_quota[qi]()
                            qi += 1
                        if prev is not None:
                            # group 0's PV, one pair per iteration of group 1
                            pv(prev[0], prev[1][st], st, prev[2])
                            if st == NST - 1:
                                tail(gi - 1, 0, prev[2], prev[0], prev[3])
                        if gi >= 1 and st >= 2:
                            pv(pctx, exs[st - 2], st - 2, pr)
                    while qi < len(ex_quota):
                        ex_quota[qi]()
                        qi += 1
                    if gi == 0:
                        prev = (pctx, exs, pr, rs)
                        continue
                    if gi == 1:
                        prev = None
                    pv(pctx, exs[NST - 2], NST - 2, pr)
                    pv(pctx, exs[NST - 1], NST - 1, pr)
                    tail(gi, tch, pr, pctx, rs)

            with nc.named_scope("outproj"):
                for tt in range(12, NST):
                    outproj(tt)
    nc.compile()
    return nc


_NC = None


def _get_nc():
    global _NC
    if _NC is None:
        _NC = build_kernel()
    return _NC


def make_in_maps(hidden_states, Wq, bq, Wk, bk, Wv, bv, Wo, bo, gate):
    f = np.float32
    b16 = ml_dtypes.bfloat16
    hidden_states = np.asarray(hidden_states, f)
    Wq, bq = np.asarray(Wq, f), np.asarray(bq, f)
    Wk, bk = np.asarray(Wk, f), np.asarray(bk, f)
    Wv, bv = np.asarray(Wv, f), np.asarray(bv, f)
    Wo, bo = np.asarray(Wo, f), np.asarray(bo, f)
    gate = np.asarray(gate, f)

    hT_b = [np.ascontiguousarray(hidden_states[b].T)
            .reshape(NEC, P, T).astype(b16) for b in range(B)]
    on2_np = np.zeros((P, 4), b16)
    on2_np[:, 0] = 1.0   # head-A rowsum -> psum row 0
    on2_np[:, 3] = 1.0   # head-B rowsum -> psum row 1
    sel2_np = np.zeros((2, P), b16)
    sel2_np[0, 0:64] = 1.0
    sel2_np[1, 64:P] = 1.0

    in_maps = []
    consts = []
    for core in range(NCORES):
        b, hg = divmod(core, 4)
        hs = [4 * hg + i for i in range(4)]

        def pack_qk(W):
            outw = np.empty((P, 2, NEC, P), f)
            for pr in range(2):
                pair = np.concatenate(
                    [W[hs[2 * pr]], W[hs[2 * pr + 1]]], axis=1)  # [E, 128]
                outw[:, pr] = pair.reshape(NEC, P, P).transpose(1, 0, 2)
            return outw.astype(b16)

        wv_np = np.concatenate([Wv[h] for h in hs], axis=1)  # [E, 256]
        wv_np = wv_np.reshape(NEC, P, 256).transpose(1, 0, 2).astype(b16)
        wo_np = np.empty((2, P, E), f)
        bq_np = np.empty((P, 2), f)
        bk_np = np.empty((P, 2), f)
        for pr in range(2):
            h0, h1 = hs[2 * pr], hs[2 * pr + 1]
            wo_np[pr] = np.concatenate(
                [gate[h0] * Wo[h0], gate[h1] * Wo[h1]], axis=0)  # [128, E]
            bq_np[:, pr] = np.concatenate([bq[h0], bq[h1]])
            bk_np[:, pr] = np.concatenate([bk[h0], bk[h1]])
        # constant term: sum_h gate_h * (bo_h + bv_h @ Wo_h)   [E]
        cst = sum(gate[h] * (bo[h] + bv[h] @ Wo[h]) for h in hs)
        consts.append(np.asarray(cst, f))
        in_maps.append(dict(
            hT=np.ascontiguousarray(hT_b[b]),
            wq=np.ascontiguousarray(pack_qk(Wq)),
            wk=np.ascontiguousarray(pack_qk(Wk)),
            wv=np.ascontiguousarray(wv_np),
            wo=np.ascontiguousarray(wo_np.transpose(1, 0, 2).astype(b16)),
            bq=bq_np, bk=bk_np,
            on2=on2_np, sel2=sel2_np,
        ))
    return in_maps, consts


def kernel(hidden_states, Wq, bq, Wk, bk, Wv, bv, Wo, bo, gate, _trace=False,
           **run_kwargs):
    nc = _get_nc()
    in_maps, consts = make_in_maps(
        hidden_states, Wq, bq, Wk, bk, Wv, bv, Wo, bo, gate)
    res = bass_utils.run_bass_kernel_spmd(
        nc, in_maps, core_ids=list(range(NCORES)), trace=_trace, **run_kwargs)
    outs = [r["out"] for r in res.results]
    full = np.stack([
        outs[0] + outs[1] + outs[2] + outs[3]
        + (consts[0] + consts[1] + consts[2] + consts[3])[None, :],
        outs[4] + outs[5] + outs[6] + outs[7]
        + (consts[4] + consts[5] + consts[6] + consts[7])[None, :],
    ]).astype(np.float32)
    kernel.last_result = res
    return full


# revision 11
# speedup vs baseline: 1.8586x; 1.0213x over previous
"""Gated multi-head self-attention on 8 Trainium2 NeuronCores.

Sharding: batch (B=2) x head-groups (4 groups of 4 heads) -> 8 cores.
Each core computes, for its batch b and its 4 heads:
    partial_out[t, e] = sum_h gate[h] * softmax(Q_h K_h^T / 8) V_h Wo_h
The host sums the 4 head-group partials per batch, adds the constant
term sum_h gate_h*(bo_h + bv_h Wo_h) (bv/bo commute past the softmax
normalization), and stacks the two batches.

v2 design (ACT exp is the critical path: 8 groups x 16 x [128,1024]
exps ~= 141us/core):
  - all matmul inputs bf16 (halves DMA, enables FWL weight loads);
    scores themselves accumulate in fp32 PSUM so softmax is accurate
  - no K=1 bias matmuls: bq/bk added during the DVE eviction of Q/K
    (per-partition scalar add), bv/bo folded into a host-side constant
  - rowsum: DVE adds ex tiles into 4 partial sums, PE accumulates the
    partials via [128,2]-ones stationary matmuls -> [2,512] PSUM row
    per head, reciprocal_approx_fast, one sel2 broadcast matmul
  - attention groups pr-outer; scores/exp stream ahead, PV lags by 2;
    V-proj, remaining Q/K projections and outproj are emitted after the
    group that needs them next, so the Tile scheduler (priority =
    emission order) runs them in PE slack under the ACT-bound groups
"""

import numpy as np
import ml_dtypes
from contextlib import ExitStack

import concourse.bass as bass
import concourse.tile as tile
from concourse import bacc, mybir
from concourse import bass_utils

E, H, D = 1024, 16, 64
B, T = 2, 2048
NCORES = 8
P = 128
TC = 512          # t-chunk (PSUM bank = 512 fp32)
NTC = T // TC     # 4 t-chunks
NST = T // P      # 16 s-tiles
NEC = E // P      # 8 e-chunks

F32 = mybir.dt.float32
F32R = mybir.dt.float32r
BF16 = mybir.dt.bfloat16
ADD = mybir.AluOpType.add
MULT = mybir.AluOpType.mult


def build_kernel():
    nc = bacc.Bacc("TRN2", target_bir_lowering=False, debug=False,
                   num_devices=NCORES)
    hT = nc.dram_tensor("hT", [NEC, P, T], BF16, kind="ExternalInput").ap()
    wq = nc.dram_tensor("wq", [P, 2, NEC, P], BF16, kind="ExternalInput").ap()
    wk = nc.dram_tensor("wk", [P, 2, NEC, P], BF16, kind="ExternalInput").ap()
    wv = nc.dram_tensor("wv", [P, NEC, 256], BF16, kind="ExternalInput").ap()
    wo = nc.dram_tensor("wo", [P, 2, E], BF16, kind="ExternalInput").ap()
    bq = nc.dram_tensor("bq", [P, 2], F32, kind="ExternalInput").ap()
    bk = nc.dram_tensor("bk", [P, 2], F32, kind="ExternalInput").ap()
    on2 = nc.dram_tensor("on2", [P, 4], BF16, kind="ExternalInput").ap()
    sel2 = nc.dram_tensor("sel2", [2, P], BF16, kind="ExternalInput").ap()
    out = nc.dram_tensor("out", [T, E], F32, kind="ExternalOutput").ap()

    with tile.TileContext(nc) as tc:
        with ExitStack() as ctx:
            persist = ctx.enter_context(tc.tile_pool(name="persist", bufs=1))
            expool = ctx.enter_context(tc.tile_pool(name="expool", bufs=20))
            rspool = ctx.enter_context(tc.tile_pool(name="rspool", bufs=2))
            rcpool = ctx.enter_context(tc.tile_pool(name="rcpool", bufs=2))
            work = ctx.enter_context(tc.tile_pool(name="work", bufs=4))
            ps_s = ctx.enter_context(tc.tile_pool(name="ps_s", bufs=2, space="PSUM"))
            ps_ctx = ctx.enter_context(tc.tile_pool(name="ps_ctx", bufs=2, space="PSUM"))
            ps_misc = ctx.enter_context(tc.tile_pool(name="ps_misc", bufs=2, space="PSUM"))

            # ---- persistent SBUF tensors ----
            hT_sb = persist.tile([P, NEC, T], BF16, tag="hT")
            wq_sb = persist.tile([P, 2, NEC, P], BF16, tag="wq")
            wk_sb = persist.tile([P, 2, NEC, P], BF16, tag="wk")
            wv_sb = persist.tile([P, NEC, 256], BF16, tag="wv")
            wo_sb = persist.tile([P, 2, E], BF16, tag="wo")
            bq_sb = persist.tile([P, 2], F32, tag="bq")
            bk_sb = persist.tile([P, 2], F32, tag="bk")
            on2_sb = persist.tile([P, 4], BF16, tag="on2")
            sel2_sb = persist.tile([2, P], BF16, tag="sel2")
            QT_sb = persist.tile([P, 2, T], BF16, tag="QT")
            KT_sb = persist.tile([P, 2, T], BF16, tag="KT")
            V_sb = persist.tile([P, NST, 256], BF16, tag="V")
            ctx_sb = persist.tile([P, 2, T], BF16, tag="ctx")

            with nc.named_scope("load"):
                # PE warm-up: ~24 dummy matmuls on uninitialized SBUF keep the
                # tensor engine busy during the input DMAs so the HAM clock
                # gate reaches 8/8 (2.4 GHz) before the real work starts.
                for w in range(8):
                    psw = ps_misc.tile([P, TC], F32, tag="ps_misc",
                                       name=f"warm{w}")
                    nc.tensor.matmul(psw[:], KT_sb[0:64, 0, 0:P],
                                     QT_sb[0:64, 0, 0:TC],
                                     start=True, stop=True)
                nc.sync.dma_start(wk_sb[:], wk)
                nc.sync.dma_start(wq_sb[:], wq)
                nc.sync.dma_start(bq_sb[:], bq)
                nc.sync.dma_start(bk_sb[:], bk)
                nc.sync.dma_start(on2_sb[:], on2)
                nc.sync.dma_start(sel2_sb[:], sel2)
                for ec in range(NEC):
                    nc.sync.dma_start(hT_sb[:, ec, :], hT[ec])
                nc.sync.dma_start(wv_sb[:], wv)
                nc.sync.dma_start(wo_sb[:], wo)

            def proj_qk(w_sb, b_sb, dst, pr, tch):
                """One [128, TC] chunk of the Q or K projection (+bias)."""
                ps = ps_misc.tile([P, TC], F32, tag="ps_misc")
                for ec in range(NEC):
                    nc.tensor.matmul(
                        ps[:], w_sb[:, pr, ec, :],
                        hT_sb[:, ec, tch * TC:(tch + 1) * TC],
                        start=(ec == 0), stop=(ec == NEC - 1))
                nc.vector.tensor_scalar(
                    dst[:, pr, tch * TC:(tch + 1) * TC], ps[:],
                    b_sb[:, pr:pr + 1], None, ADD)

            def proj_v(st):
                """V rows for s-tile st: [128 t, 256 d] -> V_sb bf16."""
                ps = ps_misc.tile([P, TC], F32, tag="ps_misc")
                psv = ps[:, :256]
                for ec in range(NEC):
                    nc.tensor.matmul(
                        psv, hT_sb[:, ec, st * P:(st + 1) * P],
                        wv_sb[:, ec, :], start=(ec == 0), stop=(ec == NEC - 1))
                nc.vector.tensor_copy(V_sb[:, st, :], psv)

            def outproj(tt):
                """Output projection for t-tile tt (128 t) -> HBM."""
                for ec2 in range(2):
                    pso = ps_misc.tile([P, TC], F32, tag="ps_misc")
                    for pr in range(2):
                        nc.tensor.matmul(
                            pso[:], ctx_sb[:, pr, tt * P:(tt + 1) * P],
                            wo_sb[:, pr, ec2 * TC:(ec2 + 1) * TC],
                            start=(pr == 0), stop=(pr == 1))
                    o_sb = work.tile([P, TC], F32, tag="o")
                    nc.vector.tensor_copy(o_sb[:], pso[:])
                    nc.sync.dma_start(
                        out[tt * P:(tt + 1) * P, ec2 * TC:(ec2 + 1) * TC],
                        o_sb[:])

            def pv(pctx, ex, st, pr):
                c0 = pr * P
                nc.tensor.matmul(
                    pctx[0:64, :], V_sb[:, st, c0:c0 + 64], ex[:, :TC],
                    start=(st == 0), stop=(st == NST - 1),
                    tile_position=(0, 0), skip_group_check=True)
                nc.tensor.matmul(
                    pctx[64:P, :], V_sb[:, st, c0 + 64:c0 + P], ex[:, TC:],
                    start=(st == 0), stop=(st == NST - 1),
                    tile_position=(0, 64), skip_group_check=True)

            with nc.named_scope("qkv"):
                proj_qk(wk_sb, bk_sb, KT_sb, 0, 0)
                proj_qk(wq_sb, bq_sb, QT_sb, 0, 0)

            # work quanta interleaved one-per-iteration into the attention
            # groups as (min_iteration, fn); emission order = semantic order
            # AND scheduler priority.  K(t1..3) are just-in-time inside
            # group 0 (scores of s-tile 4*t need K(t), first used at
            # iteration 4*t); V is split across groups 0-1 ahead of the
            # lag-4 PV consumers; outproj(tt of tch) waits for the ctx
            # scale of group 4+tch, spilled to iteration 6 of group 5+tch.
            def QK(wb, bb, dstb, pr, t):
                return lambda: proj_qk(wb, bb, dstb, pr, t)

            extras = {
                0: [(0, QK(wk_sb, bk_sb, KT_sb, 0, 1)),
                    (1, QK(wk_sb, bk_sb, KT_sb, 0, 2)),
                    (2, QK(wk_sb, bk_sb, KT_sb, 0, 3)),
                    (3, QK(wq_sb, bq_sb, QT_sb, 0, 1))]
                   + [(4 + j, (lambda st=st: proj_v(st)))
                      for j, st in enumerate(range(0, 12))],
                1: [(j, (lambda st=st: proj_v(st)))
                    for j, st in enumerate(range(12, NST))]
                   + [(4, QK(wq_sb, bq_sb, QT_sb, 0, 2)),
                      (8, QK(wk_sb, bk_sb, KT_sb, 1, 0))],
                2: [(0, QK(wq_sb, bq_sb, QT_sb, 0, 3)),
                    (4, QK(wk_sb, bk_sb, KT_sb, 1, 1)),
                    (8, QK(wk_sb, bk_sb, KT_sb, 1, 2))],
                3: [(0, QK(wk_sb, bk_sb, KT_sb, 1, 3)),
                    (6, QK(wq_sb, bq_sb, QT_sb, 1, 0))],
                4: [(0, QK(wq_sb, bq_sb, QT_sb, 1, 1))],
                5: [(0, QK(wq_sb, bq_sb, QT_sb, 1, 2))]
                   + [(7 + 2 * j, (lambda tt=tt: outproj(tt)))
                      for j, tt in enumerate(range(0, 4))],
                6: [(0, QK(wq_sb, bq_sb, QT_sb, 1, 3))]
                   + [(7 + 2 * j, (lambda tt=tt: outproj(tt)))
                      for j, tt in enumerate(range(4, 8))],
                7: [(7 + 2 * j, (lambda tt=tt: outproj(tt)))
                    for j, tt in enumerate(range(8, 12))],
            }

            # ---- attention: 8 groups, pr-outer ----
            # Group 0's PV matmuls and tail are deferred into group 1 so
            # V-proj (interleaved through group 0) has a full group of
            # margin; later groups run PV with a 2-iteration lag.
            def tail_pieces(gi, tch, pr, pctx, rs):
                t0 = tch * TC
                state = {}

                def p_rowsum():
                    prs = ps_misc.tile([P, TC], F32, tag="ps_misc",
                                       name=f"prs_{gi}")
                    nc.tensor.matmul(prs[0:2, :], on2_sb[:, 0:2], rs[:, :TC],
                                     start=True, stop=False)
                    nc.tensor.matmul(prs[0:2, :], on2_sb[:, 2:4], rs[:, TC:],
                                     start=False, stop=True)
                    rcp = rcpool.tile([2, TC], F32, tag="rcp")
                    nc.vector.reciprocal_approx_fast(rcp[:], prs[0:2, :])
                    rcp_bf = rcpool.tile([2, TC], BF16, tag="rcpb")
                    nc.vector.tensor_copy(rcp_bf[:], rcp[:])
                    state["rcp_bf"] = rcp_bf

                def p_bcast():
                    pR = ps_misc.tile([P, TC], F32, tag="ps_misc",
                                      name=f"pR_{gi}")
                    nc.tensor.matmul(pR[:], sel2_sb[:], state["rcp_bf"][:],
                                     start=True, stop=True)
                    state["pR"] = pR

                def p_mult():
                    R_sb = work.tile([P, TC], F32, tag="R", name=f"R_{gi}")
                    nc.vector.tensor_copy(R_sb[:], state["pR"][:])
                    nc.vector.tensor_tensor(
                        ctx_sb[:, pr, t0:t0 + TC], pctx[:], R_sb[:], MULT)

                return [p_rowsum, p_bcast, p_mult]

            with nc.named_scope("attn"):
                groups = [(tch, pr) for pr in range(2) for tch in range(NTC)]
                LAG = 4
                spill = []   # prev group leftovers: PV 12..15 + tail pieces
                for gi, (tch, pr) in enumerate(groups):
                    t0 = tch * TC
                    quota = sorted(extras.get(gi, []), key=lambda x: x[0])
                    qi = 0
                    pctx = ps_ctx.tile([P, TC], F32, tag="ps_ctx")
                    rs = rspool.tile([P, 2 * TC], BF16, tag="rs",
                                     name=f"rs_{gi}")
                    exs = [None] * NST
                    for st in range(NST):
                        s0 = st * P
                        pss = ps_s.tile([P, 2 * TC], F32, tag="ps_s")
                        nc.tensor.matmul(
                            pss[:, :TC], KT_sb[0:64, pr, s0:s0 + P],
                            QT_sb[0:64, pr, t0:t0 + TC],
                            start=True, stop=True, tile_position=(0, 0))
                        nc.tensor.matmul(
                            pss[:, TC:], KT_sb[64:P, pr, s0:s0 + P],
                            QT_sb[64:P, pr, t0:t0 + TC],
                            start=True, stop=True, tile_position=(64, 0))
                        ex = expool.tile([P, 2 * TC], BF16, tag="expT")
                        exs[st] = ex
                        nc.scalar.activation(
                            ex[:], pss[:],
                            mybir.ActivationFunctionType.Exp, scale=0.125)
                        if st == 1:
                            nc.vector.tensor_tensor(
                                rs[:], exs[0][:], ex[:], ADD)
                        elif st > 1:
                            nc.vector.tensor_tensor(rs[:], rs[:], ex[:], ADD)
                        # one extra-work quantum per iteration
                        while qi < len(quota) and quota[qi][0] <= st:
                            quota[qi][1]()
                            qi += 1
                            break
                        # one prev-group spill op per iteration
                        if st < len(spill):
                            spill[st]()
                        # this group's PV, lagged so its exp wait and V-proj
                        # are long satisfied when the PE reaches it
                        if st >= LAG:
                            pv(pctx, exs[st - LAG], st - LAG, pr)
                    while qi < len(quota):
                        quota[qi][1]()
                        qi += 1
                    spill = [
                        (lambda s=s, pc=pctx, e=exs[s], p=pr: pv(pc, e, s, p))
                        for s in range(NST - LAG, NST)
                    ] + tail_pieces(gi, tch, pr, pctx, rs)
                # last group's leftovers
                for fn in spill:
                    fn()

            with nc.named_scope("outproj"):
                for tt in range(12, NST):
                    outproj(tt)
    nc.compile()
    return nc


_NC = None


def _get_nc():
    global _NC
    if _NC is None:
        _NC = build_kernel()
    return _NC


def make_in_maps(hidden_states, Wq, bq, Wk, bk, Wv, bv, Wo, bo, gate):
    f = np.float32
    b16 = ml_dtypes.bfloat16
    hidden_states = np.asarray(hidden_states, f)
    Wq, bq = np.asarray(Wq, f), np.asarray(bq, f)
    Wk, bk = np.asarray(Wk, f), np.asarray(bk, f)
    Wv, bv = np.asarray(Wv, f), np.asarray(bv, f)
    Wo, bo = np.asarray(Wo, f), np.asarray(bo, f)
    gate = np.asarray(gate, f)

    hT_b = [np.ascontiguousarray(hidden_states[b].T)
            .reshape(NEC, P, T).astype(b16) for b in range(B)]
    on2_np = np.zeros((P, 4), b16)
    on2_np[:, 0] = 1.0   # head-A rowsum -> psum row 0
    on2_np[:, 3] = 1.0   # head-B rowsum -> psum row 1
    sel2_np = np.zeros((2, P), b16)
    sel2_np[0, 0:64] = 1.0
    sel2_np[1, 64:P] = 1.0

    in_maps = []
    consts = []
    for core in range(NCORES):
        b, hg = divmod(core, 4)
        hs = [4 * hg + i for i in range(4)]

        def pack_qk(W):
            outw = np.empty((P, 2, NEC, P), f)
            for pr in range(2):
                pair = np.concatenate(
                    [W[hs[2 * pr]], W[hs[2 * pr + 1]]], axis=1)  # [E, 128]
                outw[:, pr] = pair.reshape(NEC, P, P).transpose(1, 0, 2)
            return outw.astype(b16)

        wv_np = np.concatenate([Wv[h] for h in hs], axis=1)  # [E, 256]
        wv_np = wv_np.reshape(NEC, P, 256).transpose(1, 0, 2).astype(b16)
        wo_np = np.empty((2, P, E), f)
        bq_np = np.empty((P, 2), f)
        bk_np = np.empty((P, 2), f)
        for pr in range(2):
            h0, h1 = hs[2 * pr], hs[2 * pr + 1]
            wo_np[pr] = np.concatenate(
                [gate[h0] * Wo[h0], gate[h1] * Wo[h1]], axis=0)  # [128, E]
            bq_np[:, pr] = np.concatenate([bq[h0], bq[h1]])
            bk_np[:, pr] = np.concatenate([bk[h0], bk[h1]])
        # constant term: sum_h gate_h * (bo_h + bv_h @ Wo_h)   [E]
        cst = sum(gate[h] * (bo[h] + bv[h] @ Wo[h]) for h in hs)
        consts.append(np.asarray(cst, f))
        in_maps.append(dict(
            hT=np.ascontiguousarray(hT_b[b]),
            wq=np.ascontiguousarray(pack_qk(Wq)),
            wk=np.ascontiguousarray(pack_qk(Wk)),
            wv=np.ascontiguousarray(wv_np),
            wo=np.ascontiguousarray(wo_np.transpose(1, 0, 2).astype(b16)),
            bq=bq_np, bk=bk_np,
            on2=on2_np, sel2=sel2_np,
        ))
    return in_maps, consts


def kernel(hidden_states, Wq, bq, Wk, bk, Wv, bv, Wo, bo, gate, _trace=False,
           **run_kwargs):
    nc = _get_nc()
    in_maps, consts = make_in_maps(
        hidden_states, Wq, bq, Wk, bk, Wv, bv, Wo, bo, gate)
    res = bass_utils.run_bass_kernel_spmd(
        nc, in_maps, core_ids=list(range(NCORES)), trace=_trace, **run_kwargs)
    outs = [r["out"] for r in res.results]
    full = np.stack([
        outs[0] + outs[1] + outs[2] + outs[3]
        + (consts[0] + consts[1] + consts[2] + consts[3])[None, :],
        outs[4] + outs[5] + outs[6] + outs[7]
        + (consts[4] + consts[5] + consts[6] + consts[7])[None, :],
    ]).astype(np.float32)
    kernel.last_result = res
    return full
